# revision 34
# baseline (speedup 1.0000x reference)
"""Trainium2 Bass kernel for nn_Block_22832046145821 (dense_mlp).

Reference computation (B=256, D0=16, D1=32, D2=64, D_FFN=2048):
    x1 = x.reshape(B, D0, F1)                    F1 = D1*D2 = 2048
    u  = mlp1_i(x1[:, i, :]) for each i          (16 independent MLPs, hidden 2048)
    x2 = x.transpose(0,2,1,3).reshape(B, D1, F2) F2 = D0*D2 = 1024
    v  = mlp2_j(x2[:, j, :]) for each j          (32 independent MLPs, hidden 2048)
    out = x + 0.5*(u + v)

Sharding: expert-parallel across 8 cores. Core c owns mlp1 experts
{2c, 2c+1} and mlp2 experts {4c..4c+3}; every core sees the full batch.
This keeps per-core weight traffic at 1/8 of the total (33.6MB fp8), which
both the compute and HBM rooflines need (per-core HBM ~357GB/s measured).

Precision: fp8 e4m3 everywhere on the PE (weights pre-scaled by 64 into
e4m3 range on the host, x/h quantized on the fly), enabling
MatmulPerfMode.DoubleRow: each matmul contracts 256 elements (two 128-row
k-planes; lhsT [128, 2, M], rhs [128, 2, N]) at 157 TF/s -- 2x bf16. PSUM
accumulates fp32; the 64x weight scale is folded into the PSUM-drain
scale (1/64 GEMM1, 0.5/64 GEMM2). The residual `x + .` and the GEMM2
bias are applied on the host in fp32, so the dominant term never passes
through fp8. End-to-end rel err 1.63e-2 (gate 2e-2), matching the numpy
fp8 simulation exactly; error splits ~evenly between x/W/h quantization.

Device program per expert (SPMD, weight-stationary GEMM1, h-stationary
GEMM2):
  GEMM1: hT[k,b] = gelu(psum/64 + b0[k]) -> fp8 hT, k on partitions.
      Stationary = W0 slab [128, 2, DF] per 256-feature block (4 or 8
      slabs, resident across 4 phases x 4 PSUM banks); moving = xT
      [128, 2, B]. 16 matmuls/bank-group accumulate over feature blocks.
  GEMM2 (operand-swapped): stationary = hT DoubleRow block
      [128, 2, 128-batch-half]; moving = W1 slab columns [128, 2, 512].
      One instruction emits a [128-batch, 512-f] PSUM region ->
      half the instruction count of the natural orientation; drain is a
      bias-free 0.5/64 scale to bf16, b-major output.
Totals per core: 768 matmul instructions (512 GEMM1 @ 256 moving + 256
GEMM2 @ 512 moving) = 109.3us of PE floor at 2.4GHz.

Schedule notes (profile-driven):
- All weight slabs stream on the SP DGE queue in need order; x/bias/out
  on the ACT queue. Never put output DMAs on the weight queue (head-of-
  line blocking behind drain-dependent stores measured +25us).
- w2ring=18 so the next expert's GEMM2 slabs never wait on ring WAR.
- GEMM2 drains on ACT, not DVE: concurrent DVE PSUM reads slow the
  matmul stream ~17% (PSUM port contention). DVE is used only in the
  final kernel phase (no matmuls left to disturb), alternating with ACT.
- Final phase completes one PSUM region at a time (region-outer) and
  stores in two halves so the post-matmul tail is ~3us, not ~6.
- Cold start: slab0 on the ACT queue in parallel with xt on the SP
  queue; first expert is an mlp2 (4 slabs, halves the fill stall).
- Framework preamble (~8.5us to first DMA trigger) and teardown (~1.7us)
  are fixed; cold fill is HBM-bandwidth-bound, not latency-bound.

Measured (8x trn2 NeuronCores via axon): HW exec ~137.5-139us
(baseline bf16 version of the same schedule: 267us; PE span ~116us at
113ns/instr incl. ~8-13us of clock throttle; weight DMA 94us active at
357GB/s). Run-to-run/compile-to-compile noise is 1-3us; machine-state
drift across a session is a further few us -- A/B only back-to-back.
"""

import sys
from concurrent.futures import ThreadPoolExecutor

import numpy as np

try:
    import concourse.bass as bass
except ImportError:  # pragma: no cover
    sys.path.insert(0, "/opt/trn_rl_repo")
    import concourse.bass as bass

import ml_dtypes
import concourse.mybir as mybir
from concourse import bacc
from concourse.bass_utils import run_bass_kernel_spmd
from concourse.tile import TileContext

B, D0, D1, D2 = 256, 16, 32, 64
DF = 2048
F1 = D1 * D2  # 2048
F2 = D0 * D2  # 1024
NCORES = 8
E1 = D0 // NCORES  # 2 mlp1 experts per core
E2 = D1 // NCORES  # 4 mlp2 experts per core
KT = DF // 128  # 16 hidden tiles
WSCALE = 64.0   # weight pre-scale into e4m3 range

BF16 = mybir.dt.bfloat16
F32 = mybir.dt.float32
FP8 = mybir.dt.float8e4
NPBF16 = ml_dtypes.bfloat16
NPFP8 = ml_dtypes.float8_e4m3

GELU = mybir.ActivationFunctionType.Gelu
IDENT = mybir.ActivationFunctionType.Identity
DBLROW = mybir.MatmulPerfMode.DoubleRow

_PROGRAM = None


class _Ring:
    """Explicit round-robin ring of SBUF tiles."""

    def __init__(self, pool, shape, dtype, n, name):
        self.tiles = [
            pool.tile(shape, dtype, name=f"{name}{i}", tag=f"{name}{i}")
            for i in range(n)
        ]
        self.idx = 0

    def acquire(self):
        i = self.idx % len(self.tiles)
        self.idx += 1
        return self.tiles[i]


def _emit_loads(nc, rings, spec, xt_pre=None):
    """Input DMAs for one expert: xT (one transfer) + combined bias, on the
    ACT engine's DGE queue so SP stays free for weight-slab issue. xt_pre
    passes an already-loading xt tile (cold-start fast fill)."""
    xring, wring, w2ring, hring, bpool, oring, pspool = rings
    xt_dram, bb_dram, e, F, tag = (
        spec["xt"], spec["bb"], spec["e"], spec["F"], spec["tag"])
    FT = F // 128
    if xt_pre is None:
        xt = xring.acquire()
        nc.scalar.dma_start(out=xt[:, :FT, :], in_=xt_dram[e])
    else:
        xt = xt_pre
    bb = bpool.tile([128, KT], F32, tag=f"bb_{tag}_{e}")
    nc.scalar.dma_start(out=bb[:], in_=bb_dram[e])
    return {"xt": xt, "b0": bb[:, :KT]}


def _emit_expert_mlp(nc, rings, spec, loads, next_loads_fn):
    """One expert MLP: [F] -> gelu -> [DF] -> [F], batch B, transposed
    layout, fp8 DoubleRow matmuls (256-element contraction per instr).

    spec tensors (per expert e), partition-major packing done on host:
      xt:  [E, 128, F//128, B] fp8   xT      w0t: [E, F//256, 128, 2, DF] fp8
      bb:  [E, 128, KT + F//128] f32         w1t: [E, KT//2, 128, 2, F] fp8
      out: [E, F//512, 128, 4, B] bf16 (0.5*y.T, phase-batched)
    w0t[e,fb,p,i,k] = 64*W0[k, fb*256+i*128+p]; w1t[e,kb,p,i,f] =
    64*W1[f, kb*256+i*128+p]: slab[:, :, m*128:(m+1)*128] is one DoubleRow
    lhsT (dim1 = the two k-planes).
    """
    xring, wring, w2ring, hring, bpool, oring, pspool = rings
    w0t_dram, w1t_dram, out_dram, e, F = (
        spec["w0t"], spec["w1t"], spec["out"], spec["e"], spec["F"])
    FT = F // 128   # 16 (mlp1) or 8 (mlp2)
    FB = FT // 2    # GEMM1 DoubleRow k-blocks: 8 (mlp1) or 4 (mlp2)
    KB = KT // 2    # GEMM2 DoubleRow k-blocks: 8
    xt, b0 = loads["xt"], loads["b0"]
    ht = hring.acquire()

    # GEMM1: one [128, 2, DF] fp8 slab per 256-feature contraction block
    # (both DoubleRow k-planes in the free dim), resident across all
    # phases; matmuls slice the output column range.
    slabs = spec.pop("pre_slabs", None)
    if slabs is None:
        slabs = []
        for fb in range(FB):
            slab = wring.acquire()
            nc.sync.dma_start(out=slab[:], in_=w0t_dram[e, fb])
            slabs.append(slab)
    for q in range(KT // 4):  # 4 phases x 4 PSUM banks
        ps = [pspool.tile([128, 512], F32, tag="ps", name=f"ps{i}") for i in range(4)]
        for fb in range(FB):
            for k4 in range(4):
                m = q * 4 + k4
                nc.tensor.matmul(
                    ps[k4][:, :B],
                    lhsT=slabs[fb][:, :, m * 128:(m + 1) * 128],
                    rhs=xt[:, 2 * fb:2 * fb + 2, :],
                    start=(fb == 0),
                    stop=(fb == FB - 1),
                    perf_mode=DBLROW,
                )
        for k4 in range(4):
            m = q * 4 + k4
            nc.scalar.activation(
                ht[:, m, :], ps[k4][:, :B], GELU, bias=b0[:, m:m + 1],
                scale=1.0 / WSCALE,
            )

    # Prefetch the next expert's inputs now: the xt ring slot was released
    # by this expert's last GEMM1 matmul, so the load overlaps all of GEMM2.
    next_loads = next_loads_fn() if next_loads_fn is not None else None

    # GEMM2 (operand-swapped): stationary = a DoubleRow block of hT
    # (reused across 512 moving columns), moving = W1 slab columns. Each
    # instruction emits a [128 batch, 512 f] PSUM region, so GEMM2 needs
    # half the instructions of the natural orientation; out is b-major and
    # the +0.5*b1 bias moves to the host unshard.
    slabs2 = []
    for kb in range(KB):
        slab = w2ring.acquire()
        nc.sync.dma_start(out=slab[:, :, :F], in_=w1t_dram[e, kb])
        slabs2.append(slab)
    last = spec.get("last", False)
    NF = F // 512   # f-chunks: 4 (mlp1) or 2 (mlp2)
    NB = B // 128   # 2 batch halves
    regions = [(bh, fc) for bh in range(NB) for fc in range(NF)]
    nphase = len(regions) // 4  # 2 (mlp1) or 1 (mlp2)
    for p in range(nphase):
        ps = [pspool.tile([128, 512], F32, tag="ps", name=f"ps{i}")
              for i in range(4)]
        rs = regions[p * 4:(p + 1) * 4]
        tail = last and p == nphase - 1
        if not tail:
            for kb in range(KB):
                for j, (bh, fc) in enumerate(rs):
                    nc.tensor.matmul(
                        ps[j][:],
                        lhsT=ht[:, 2 * kb:2 * kb + 2, bh * 128:(bh + 1) * 128],
                        rhs=slabs2[kb][:, :, fc * 512:(fc + 1) * 512],
                        start=(kb == 0),
                        stop=(kb == KB - 1),
                        perf_mode=DBLROW,
                    )
            ot = oring.acquire()
            for j in range(4):
                nc.scalar.activation(
                    ot[:, j, :], ps[j][:], IDENT, scale=0.5 / WSCALE,
                )
            nc.scalar.dma_start(out=out_dram[e, p], in_=ot[:])
        else:
            # Final phase of the whole kernel: complete one region at a
            # time and drain it (alternating ACT/DVE) while the next
            # region's matmuls run, so the tail after the last matmul is
            # one region deep, not four.
            ot = oring.acquire()
            for j, (bh, fc) in enumerate(rs):
                for kb in range(KB):
                    nc.tensor.matmul(
                        ps[j][:],
                        lhsT=ht[:, 2 * kb:2 * kb + 2, bh * 128:(bh + 1) * 128],
                        rhs=slabs2[kb][:, :, fc * 512:(fc + 1) * 512],
                        start=(kb == 0),
                        stop=(kb == KB - 1),
                        perf_mode=DBLROW,
                    )
                if j % 2 == 0:
                    nc.scalar.activation(
                        ot[:, j, :], ps[j][:], IDENT, scale=0.5 / WSCALE,
                    )
                else:
                    nc.vector.tensor_scalar(
                        ot[:, j, :], ps[j][:], 0.5 / WSCALE, None,
                        mybir.AluOpType.mult,
                    )
                if j == 1:
                    nc.scalar.dma_start(
                        out=out_dram[e, p, :, 0:2], in_=ot[:, 0:2, :])
                elif j == 2:
                    nc.sync.dma_start(
                        out=out_dram[e, p, :, 2:3], in_=ot[:, 2:3, :])
            nc.scalar.dma_start(out=out_dram[e, p, :, 3:4], in_=ot[:, 3:4, :])
    return next_loads


def _build_program():
    nc = bacc.Bacc()

    xt1 = nc.dram_tensor("xt1", [E1, 128, F1 // 128, B], FP8, kind="ExternalInput")
    w0t1 = nc.dram_tensor("w0t1", [E1, F1 // 256, 128, 2, DF], FP8,
                          kind="ExternalInput")
    w1t1 = nc.dram_tensor("w1t1", [E1, KT // 2, 128, 2, F1], FP8,
                          kind="ExternalInput")
    bb1 = nc.dram_tensor("bb1", [E1, 128, KT], F32, kind="ExternalInput")
    xt2 = nc.dram_tensor("xt2", [E2, 128, F2 // 128, B], FP8, kind="ExternalInput")
    w0t2 = nc.dram_tensor("w0t2", [E2, F2 // 256, 128, 2, DF], FP8,
                          kind="ExternalInput")
    w1t2 = nc.dram_tensor("w1t2", [E2, KT // 2, 128, 2, F2], FP8,
                          kind="ExternalInput")
    bb2 = nc.dram_tensor("bb2", [E2, 128, KT], F32, kind="ExternalInput")
    outU = nc.dram_tensor("outU", [E1, 2, 128, 4, 512], BF16,
                          kind="ExternalOutput")
    outV = nc.dram_tensor("outV", [E2, 1, 128, 4, 512], BF16,
                          kind="ExternalOutput")

    specs_u = [
        {"xt": xt1, "w0t": w0t1, "w1t": w1t1, "bb": bb1,
         "out": outU, "e": e, "F": F1, "tag": "u"}
        for e in range(E1)
    ]
    specs_v = [
        {"xt": xt2, "w0t": w0t2, "w1t": w1t2, "bb": bb2,
         "out": outV, "e": e, "F": F2, "tag": "v"}
        for e in range(E2)
    ]
    # Start with an mlp2 expert: its GEMM1 needs only 4 slabs, so the
    # cold-start fill stall is half as long as an mlp1 expert's.
    specs = [specs_v[0]] + specs_u + specs_v[1:]
    specs[-1]["last"] = True

    with TileContext(nc) as tc:
        with (
            tc.tile_pool(name="xp", bufs=1) as xpool,
            tc.tile_pool(name="wp", bufs=1) as wpool,
            tc.tile_pool(name="hp", bufs=1) as hpool,
            tc.tile_pool(name="bp", bufs=1) as bpool,
            tc.tile_pool(name="op", bufs=1) as opool,
            tc.tile_pool(name="pp", bufs=8, space="PSUM") as pspool,
        ):
            xring = _Ring(xpool, [128, F1 // 128, B], FP8, 2, "xt")
            wring = _Ring(wpool, [128, 2, DF], FP8, 14, "w")
            w2ring = _Ring(wpool, [128, 2, F1], FP8, 18, "w2")
            hring = _Ring(hpool, [128, KT, B], FP8, 2, "ht")
            oring = _Ring(opool, [128, 4, 512], BF16, 4, "ot")
            rings = (xring, wring, w2ring, hring, bpool, oring, pspool)

            # Cold start: the first matmul needs slab0 + xt; put slab0 on
            # the ACT queue and xt on the SP queue so they land in
            # parallel, with slab1 prefetched behind slab0.
            pre = [wring.acquire() for _ in range(2)]
            nc.scalar.dma_start(out=pre[0][:], in_=specs[0]["w0t"][0, 0])
            xt0 = xring.acquire()
            nc.sync.dma_start(
                out=xt0[:, :F2 // 128, :], in_=specs[0]["xt"][0])
            loads = _emit_loads(nc, rings, specs[0], xt_pre=xt0)
            nc.scalar.dma_start(out=pre[1][:], in_=specs[0]["w0t"][0, 1])
            for fb in range(2, F2 // 256):
                slab = wring.acquire()
                nc.sync.dma_start(out=slab[:], in_=specs[0]["w0t"][0, fb])
                pre.append(slab)
            specs[0]["pre_slabs"] = pre
            for i, spec in enumerate(specs):
                if i + 1 < len(specs):
                    nl_fn = (lambda s=specs[i + 1]: _emit_loads(nc, rings, s))
                else:
                    nl_fn = None
                nxt = _emit_expert_mlp(nc, rings, spec, loads, nl_fn)
                loads = nxt

    nc.finalize()
    return nc


def _get_program():
    global _PROGRAM
    if _PROGRAM is None:
        _PROGRAM = _build_program()
    return _PROGRAM


def _part_major(b, n_tiles):
    # [E, n_tiles*128] f32 -> [E, 128, n_tiles], partition-major bias layout
    e = b.shape[0]
    return np.ascontiguousarray(
        b.reshape(e, n_tiles, 128).transpose(0, 2, 1)).astype(np.float32)


def _pack_xt(xs):
    # [B, E, F] -> [E, 128, F//128, B] (partition-major xT), fp8
    Bn, En, Fn = xs.shape
    xq = xs.astype(NPFP8)
    xt = xq.transpose(1, 2, 0).reshape(En, Fn // 128, 128, Bn)
    return np.ascontiguousarray(xt.transpose(0, 2, 1, 3))


def _pack_w0(W0):
    # [E, DF, F] f32 -> [E, F//256, 128, 2, DF] fp8 where
    # out[e, fb, p, i, k] = 64*W0[e, k, fb*256 + i*128 + p]
    E, _, F = W0.shape
    Wq = (W0 * WSCALE).astype(NPFP8)
    Wv = Wq.reshape(E, DF, F // 256, 2, 128).transpose(0, 2, 4, 3, 1)
    return np.ascontiguousarray(Wv)


def _pack_w1(W1):
    # [E, F, DF] f32 -> [E, KT//2, 128, 2, F] fp8 where
    # out[e, kb, p, i, f] = 64*W1[e, f, kb*256 + i*128 + p]
    E, F, _ = W1.shape
    Wq = (W1 * WSCALE).astype(NPFP8)
    Wv = Wq.reshape(E, F, KT // 2, 2, 128).transpose(0, 2, 4, 3, 1)
    return np.ascontiguousarray(Wv)


def _pack_core(c, x1, x2, W0_1, b0_1, W1_1, b1_1, W0_2, b0_2, W1_2, b1_2):
    i0, j0 = c * E1, c * E2
    s1, s2 = slice(i0, i0 + E1), slice(j0, j0 + E2)
    bb1 = _part_major(b0_1[s1], KT)
    bb2 = _part_major(b0_2[s2], KT)
    return {
        "xt1": _pack_xt(x1[:, s1, :]),
        "w0t1": _pack_w0(W0_1[s1]),
        "w1t1": _pack_w1(W1_1[s1]),
        "bb1": np.ascontiguousarray(bb1),
        "xt2": _pack_xt(x2[:, s2, :]),
        "w0t2": _pack_w0(W0_2[s2]),
        "w1t2": _pack_w1(W1_2[s2]),
        "bb2": np.ascontiguousarray(bb2),
    }


def run(inputs, trace=False):
    """Returns (out, BassKernelResults)."""
    x = np.asarray(inputs["x"], dtype=np.float32)
    x1 = x.reshape(B, D0, F1)
    x2 = np.ascontiguousarray(x.transpose(0, 2, 1, 3)).reshape(B, D1, F2)
    args = tuple(
        np.asarray(inputs[k], dtype=np.float32)
        for k in ("W0_1", "b0_1", "W1_1", "b1_1", "W0_2", "b0_2", "W1_2", "b1_2")
    )

    with ThreadPoolExecutor(max_workers=NCORES) as ex:
        in_maps = list(ex.map(lambda c: _pack_core(c, x1, x2, *args), range(NCORES)))
    nc = _get_program()
    res = run_bass_kernel_spmd(nc, in_maps, list(range(NCORES)), trace=trace)

    # outU [E, 2, 128, 4, 512]: region (p, j) = batch-half p, f-chunk j
    # -> u'[e, b, f]. outV [E, 1, 128, 4, 512]: region j = (bh, fc) with
    # fc = j % 2 -> v'[e, b, f]. Biases were dropped on-device; add the
    # 0.5*b1 terms here in fp32.
    U = np.concatenate([r["outU"] for r in res.results], axis=0).astype(np.float32)
    V = np.concatenate([r["outV"] for r in res.results], axis=0).astype(np.float32)
    U = U.transpose(0, 1, 3, 2, 4).reshape(D0, B, F1)
    V = V.reshape(D1, 128, 2, 2, 512).transpose(0, 2, 1, 3, 4).reshape(D1, B, F2)
    U = U + 0.5 * np.asarray(inputs["b1_1"], dtype=np.float32)[:, None, :]
    V = V + 0.5 * np.asarray(inputs["b1_2"], dtype=np.float32)[:, None, :]
    u_half = U.transpose(1, 0, 2).reshape(B, D0, D1, D2)
    v_half = V.transpose(1, 0, 2).reshape(B, D1, D0, D2).transpose(0, 2, 1, 3)
    out = x + u_half + v_half
    return np.ascontiguousarray(out, dtype=np.float32), res


def kernel(**inputs) -> np.ndarray:
    out, _ = run(inputs, trace=False)
    return out


# revision 35
# speedup vs baseline: 1.0555x; 1.0555x over previous
"""Trainium2 Bass kernel for nn_Block_22832046145821 (dense_mlp).

Reference computation (B=256, D0=16, D1=32, D2=64, D_FFN=2048):
    x1 = x.reshape(B, D0, F1)                    F1 = D1*D2 = 2048
    u  = mlp1_i(x1[:, i, :]) for each i          (16 independent MLPs, hidden 2048)
    x2 = x.transpose(0,2,1,3).reshape(B, D1, F2) F2 = D0*D2 = 1024
    v  = mlp2_j(x2[:, j, :]) for each j          (32 independent MLPs, hidden 2048)
    out = x + 0.5*(u + v)

Sharding: expert-parallel across 8 cores. Core c owns mlp1 experts
{2c, 2c+1} and mlp2 experts {4c..4c+3}; every core sees the full batch.
This keeps per-core weight traffic at 1/8 of the total (33.6MB fp8), which
both the compute and HBM rooflines need (per-core HBM ~357GB/s measured).

Precision: fp8 e4m3 everywhere on the PE (weights pre-scaled by 64 into
e4m3 range on the host, x/h quantized on the fly), enabling
MatmulPerfMode.DoubleRow: each matmul contracts 256 elements (two 128-row
k-planes; lhsT [128, 2, M], rhs [128, 2, N]) at 157 TF/s -- 2x bf16. PSUM
accumulates fp32; the 64x weight scale is folded into the PSUM-drain
scale (1/64 GEMM1, 0.5/64 GEMM2). The residual `x + .` and the GEMM2
bias are applied on the host in fp32, so the dominant term never passes
through fp8. End-to-end rel err 1.63e-2 (gate 2e-2), matching the numpy
fp8 simulation exactly; error splits ~evenly between x/W/h quantization.

Device program per expert (SPMD, weight-stationary GEMM1, h-stationary
GEMM2):
  GEMM1: hT[k,b] = gelu(psum/64 + b0[k]) -> fp8 hT, k on partitions.
      Stationary = W0 slab [128, 2, DF] per 256-feature block (4 or 8
      slabs, resident across 4 phases x 4 PSUM banks); moving = xT
      [128, 2, B]. 16 matmuls/bank-group accumulate over feature blocks.
  GEMM2 (operand-swapped): stationary = hT DoubleRow block
      [128, 2, 128-batch-half]; moving = W1 slab columns [128, 2, 512].
      One instruction emits a [128-batch, 512-f] PSUM region ->
      half the instruction count of the natural orientation; drain is a
      bias-free 0.5/64 scale to bf16, b-major output.
Totals per core: 768 matmul instructions (512 GEMM1 @ 256 moving + 256
GEMM2 @ 512 moving) = 109.3us of PE floor at 2.4GHz.

Schedule notes (profile-driven):
- All weight slabs stream on the SP DGE queue in need order; x/bias/out
  on the ACT queue. Never put output DMAs on the weight queue (head-of-
  line blocking behind drain-dependent stores measured +25us).
- w2ring=18 so the next expert's GEMM2 slabs never wait on ring WAR.
- GEMM2 drains on ACT, not DVE: concurrent DVE PSUM reads slow the
  matmul stream ~17% (PSUM port contention). DVE is used only in the
  final kernel phase (no matmuls left to disturb), alternating with ACT.
- Final phase completes one PSUM region at a time (region-outer) and
  stores in two halves so the post-matmul tail is ~3us, not ~6.
- Cold start: slab0 on the ACT queue in parallel with xt on the SP
  queue; first expert is an mlp2 (4 slabs, halves the fill stall).
- Framework preamble (~8.5us to first DMA trigger) and teardown (~1.7us)
  are fixed; cold fill is HBM-bandwidth-bound, not latency-bound.

Measured (8x trn2 NeuronCores via axon): HW exec ~137.5-139us
(baseline bf16 version of the same schedule: 267us; PE span ~116us at
113ns/instr incl. ~8-13us of clock throttle; weight DMA 94us active at
357GB/s). Run-to-run/compile-to-compile noise is 1-3us; machine-state
drift across a session is a further few us -- A/B only back-to-back.
"""

import sys
from concurrent.futures import ThreadPoolExecutor

import numpy as np

try:
    import concourse.bass as bass
except ImportError:  # pragma: no cover
    sys.path.insert(0, "/opt/trn_rl_repo")
    import concourse.bass as bass

import ml_dtypes
import concourse.mybir as mybir
from concourse import bacc
from concourse.bass_utils import run_bass_kernel_spmd
from concourse.tile import TileContext

B, D0, D1, D2 = 256, 16, 32, 64
DF = 2048
F1 = D1 * D2  # 2048
F2 = D0 * D2  # 1024
NCORES = 8
E1 = D0 // NCORES  # 2 mlp1 experts per core
E2 = D1 // NCORES  # 4 mlp2 experts per core
KT = DF // 128  # 16 hidden tiles
WSCALE = 64.0   # weight pre-scale into e4m3 range

BF16 = mybir.dt.bfloat16
F32 = mybir.dt.float32
FP8 = mybir.dt.float8e4
NPBF16 = ml_dtypes.bfloat16
NPFP8 = ml_dtypes.float8_e4m3

GELU = mybir.ActivationFunctionType.Gelu
IDENT = mybir.ActivationFunctionType.Identity
DBLROW = mybir.MatmulPerfMode.DoubleRow

_PROGRAM = None


class _Ring:
    """Explicit round-robin ring of SBUF tiles."""

    def __init__(self, pool, shape, dtype, n, name):
        self.tiles = [
            pool.tile(shape, dtype, name=f"{name}{i}", tag=f"{name}{i}")
            for i in range(n)
        ]
        self.idx = 0

    def acquire(self):
        i = self.idx % len(self.tiles)
        self.idx += 1
        return self.tiles[i]


def _emit_loads(nc, rings, spec, xt_pre=None):
    """Input DMAs for one expert: xT (one transfer) + combined bias, on the
    ACT engine's DGE queue so SP stays free for weight-slab issue. xt_pre
    passes an already-loading xt tile (cold-start fast fill)."""
    xring, wring, w2ring, hring, bpool, oring, pspool = rings
    xt_dram, bb_dram, e, F, tag = (
        spec["xt"], spec["bb"], spec["e"], spec["F"], spec["tag"])
    FT = F // 128
    if xt_pre is None:
        xt = xring.acquire()
        nc.scalar.dma_start(out=xt[:, :FT, :], in_=xt_dram[e])
    else:
        xt = xt_pre
    bb = bpool.tile([128, KT], F32, tag=f"bb_{tag}_{e}")
    nc.scalar.dma_start(out=bb[:], in_=bb_dram[e])
    return {"xt": xt, "b0": bb[:, :KT]}


def _emit_expert_mlp(nc, rings, spec, loads, next_loads_fn):
    """One expert MLP: [F] -> gelu -> [DF] -> [F], batch B, transposed
    layout, fp8 DoubleRow matmuls (256-element contraction per instr).

    spec tensors (per expert e), partition-major packing done on host:
      xt:  [E, 128, F//128, B] fp8   xT      w0t: [E, F//256, 128, 2, DF] fp8
      bb:  [E, 128, KT + F//128] f32         w1t: [E, KT//2, 128, 2, F] fp8
      out: [E, F//512, 128, 4, B] bf16 (0.5*y.T, phase-batched)
    w0t[e,fb,p,i,k] = 64*W0[k, fb*256+i*128+p]; w1t[e,kb,p,i,f] =
    64*W1[f, kb*256+i*128+p]: slab[:, :, m*128:(m+1)*128] is one DoubleRow
    lhsT (dim1 = the two k-planes).
    """
    xring, wring, w2ring, hring, bpool, oring, pspool = rings
    w0t_dram, w1t_dram, out_dram, e, F = (
        spec["w0t"], spec["w1t"], spec["out"], spec["e"], spec["F"])
    FT = F // 128   # 16 (mlp1) or 8 (mlp2)
    FB = FT // 2    # GEMM1 DoubleRow k-blocks: 8 (mlp1) or 4 (mlp2)
    KB = KT // 2    # GEMM2 DoubleRow k-blocks: 8
    xt, b0 = loads["xt"], loads["b0"]
    ht = hring.acquire()

    # GEMM1: one [128, 2, DF] fp8 slab per 256-feature contraction block
    # (both DoubleRow k-planes in the free dim), resident across all
    # phases; matmuls slice the output column range.
    slabs = spec.pop("pre_slabs", None)
    if slabs is None:
        slabs = []
        for fb in range(FB):
            slab = wring.acquire()
            nc.sync.dma_start(out=slab[:], in_=w0t_dram[e, fb])
            slabs.append(slab)
    for q in range(KT // 4):  # 4 phases x 4 PSUM banks
        ps = [pspool.tile([128, 512], F32, tag="ps", name=f"ps{i}") for i in range(4)]
        for fb in range(FB):
            for k4 in range(4):
                m = q * 4 + k4
                nc.tensor.matmul(
                    ps[k4][:, :B],
                    lhsT=slabs[fb][:, :, m * 128:(m + 1) * 128],
                    rhs=xt[:, 2 * fb:2 * fb + 2, :],
                    start=(fb == 0),
                    stop=(fb == FB - 1),
                    perf_mode=DBLROW,
                )
        for k4 in range(4):
            m = q * 4 + k4
            nc.scalar.activation(
                ht[:, m, :], ps[k4][:, :B], GELU, bias=b0[:, m:m + 1],
                scale=1.0 / WSCALE,
            )

    # Prefetch the next expert's inputs now: the xt ring slot was released
    # by this expert's last GEMM1 matmul, so the load overlaps all of GEMM2.
    next_loads = next_loads_fn() if next_loads_fn is not None else None

    # GEMM2 (operand-swapped): stationary = a DoubleRow block of hT
    # (reused across 512 moving columns), moving = W1 slab columns. Each
    # instruction emits a [128 batch, 512 f] PSUM region, so GEMM2 needs
    # half the instructions of the natural orientation; out is b-major and
    # the +0.5*b1 bias moves to the host unshard.
    slabs2 = []
    for kb in range(KB):
        slab = w2ring.acquire()
        nc.sync.dma_start(out=slab[:, :, :F], in_=w1t_dram[e, kb])
        slabs2.append(slab)
    last = spec.get("last", False)
    NF = F // 512   # f-chunks: 4 (mlp1) or 2 (mlp2)
    NB = B // 128   # 2 batch halves
    regions = [(bh, fc) for bh in range(NB) for fc in range(NF)]
    nphase = len(regions) // 4  # 2 (mlp1) or 1 (mlp2)
    for p in range(nphase):
        ps = [pspool.tile([128, 512], F32, tag="ps", name=f"ps{i}")
              for i in range(4)]
        rs = regions[p * 4:(p + 1) * 4]
        tail = last and p == nphase - 1
        if not tail:
            for kb in range(KB):
                for j, (bh, fc) in enumerate(rs):
                    nc.tensor.matmul(
                        ps[j][:],
                        lhsT=ht[:, 2 * kb:2 * kb + 2, bh * 128:(bh + 1) * 128],
                        rhs=slabs2[kb][:, :, fc * 512:(fc + 1) * 512],
                        start=(kb == 0),
                        stop=(kb == KB - 1),
                        perf_mode=DBLROW,
                    )
            ot = oring.acquire()
            for j in range(4):
                nc.scalar.activation(
                    ot[:, j, :], ps[j][:], IDENT, scale=0.5 / WSCALE,
                )
            nc.scalar.dma_start(out=out_dram[e, p], in_=ot[:])
        else:
            # Final phase of the whole kernel: complete one region at a
            # time and drain it (alternating ACT/DVE) while the next
            # region's matmuls run, so the tail after the last matmul is
            # one region deep, not four.
            ot = oring.acquire()
            for j, (bh, fc) in enumerate(rs):
                for kb in range(KB):
                    nc.tensor.matmul(
                        ps[j][:],
                        lhsT=ht[:, 2 * kb:2 * kb + 2, bh * 128:(bh + 1) * 128],
                        rhs=slabs2[kb][:, :, fc * 512:(fc + 1) * 512],
                        start=(kb == 0),
                        stop=(kb == KB - 1),
                        perf_mode=DBLROW,
                    )
                if j % 2 == 0:
                    nc.scalar.activation(
                        ot[:, j, :], ps[j][:], IDENT, scale=0.5 / WSCALE,
                    )
                else:
                    nc.vector.tensor_scalar(
                        ot[:, j, :], ps[j][:], 0.5 / WSCALE, None,
                        mybir.AluOpType.mult,
                    )
                if j == 1:
                    nc.scalar.dma_start(
                        out=out_dram[e, p, :, 0:2], in_=ot[:, 0:2, :])
            nc.scalar.dma_start(out=out_dram[e, p, :, 2:4], in_=ot[:, 2:4, :])
    return next_loads


def _build_program():
    nc = bacc.Bacc()

    xt1 = nc.dram_tensor("xt1", [E1, 128, F1 // 128, B], FP8, kind="ExternalInput")
    w0t1 = nc.dram_tensor("w0t1", [E1, F1 // 256, 128, 2, DF], FP8,
                          kind="ExternalInput")
    w1t1 = nc.dram_tensor("w1t1", [E1, KT // 2, 128, 2, F1], FP8,
                          kind="ExternalInput")
    bb1 = nc.dram_tensor("bb1", [E1, 128, KT], F32, kind="ExternalInput")
    xt2 = nc.dram_tensor("xt2", [E2, 128, F2 // 128, B], FP8, kind="ExternalInput")
    w0t2 = nc.dram_tensor("w0t2", [E2, F2 // 256, 128, 2, DF], FP8,
                          kind="ExternalInput")
    w1t2 = nc.dram_tensor("w1t2", [E2, KT // 2, 128, 2, F2], FP8,
                          kind="ExternalInput")
    bb2 = nc.dram_tensor("bb2", [E2, 128, KT], F32, kind="ExternalInput")
    outU = nc.dram_tensor("outU", [E1, 2, 128, 4, 512], BF16,
                          kind="ExternalOutput")
    outV = nc.dram_tensor("outV", [E2, 1, 128, 4, 512], BF16,
                          kind="ExternalOutput")

    specs_u = [
        {"xt": xt1, "w0t": w0t1, "w1t": w1t1, "bb": bb1,
         "out": outU, "e": e, "F": F1, "tag": "u"}
        for e in range(E1)
    ]
    specs_v = [
        {"xt": xt2, "w0t": w0t2, "w1t": w1t2, "bb": bb2,
         "out": outV, "e": e, "F": F2, "tag": "v"}
        for e in range(E2)
    ]
    # Start with an mlp2 expert: its GEMM1 needs only 4 slabs, so the
    # cold-start fill stall is half as long as an mlp1 expert's.
    specs = [specs_v[0]] + specs_u + specs_v[1:]
    specs[-1]["last"] = True

    with TileContext(nc) as tc:
        with (
            tc.tile_pool(name="xp", bufs=1) as xpool,
            tc.tile_pool(name="wp", bufs=1) as wpool,
            tc.tile_pool(name="hp", bufs=1) as hpool,
            tc.tile_pool(name="bp", bufs=1) as bpool,
            tc.tile_pool(name="op", bufs=1) as opool,
            tc.tile_pool(name="pp", bufs=8, space="PSUM") as pspool,
        ):
            xring = _Ring(xpool, [128, F1 // 128, B], FP8, 2, "xt")
            wring = _Ring(wpool, [128, 2, DF], FP8, 14, "w")
            w2ring = _Ring(wpool, [128, 2, F1], FP8, 18, "w2")
            hring = _Ring(hpool, [128, KT, B], FP8, 2, "ht")
            oring = _Ring(opool, [128, 4, 512], BF16, 4, "ot")
            rings = (xring, wring, w2ring, hring, bpool, oring, pspool)

            # Cold start: the first matmul needs slab0 + xt; put slab0 on
            # the ACT queue and xt on the SP queue so they land in
            # parallel, with slab1 prefetched behind slab0.
            pre = [wring.acquire() for _ in range(2)]
            nc.scalar.dma_start(out=pre[0][:], in_=specs[0]["w0t"][0, 0])
            xt0 = xring.acquire()
            nc.sync.dma_start(
                out=xt0[:, :F2 // 128, :], in_=specs[0]["xt"][0])
            loads = _emit_loads(nc, rings, specs[0], xt_pre=xt0)
            nc.scalar.dma_start(out=pre[1][:], in_=specs[0]["w0t"][0, 1])
            for fb in range(2, F2 // 256):
                slab = wring.acquire()
                nc.sync.dma_start(out=slab[:], in_=specs[0]["w0t"][0, fb])
                pre.append(slab)
            specs[0]["pre_slabs"] = pre
            for i, spec in enumerate(specs):
                if i + 1 < len(specs):
                    nl_fn = (lambda s=specs[i + 1]: _emit_loads(nc, rings, s))
                else:
                    nl_fn = None
                nxt = _emit_expert_mlp(nc, rings, spec, loads, nl_fn)
                loads = nxt

    nc.finalize()
    return nc


def _get_program():
    global _PROGRAM
    if _PROGRAM is None:
        _PROGRAM = _build_program()
    return _PROGRAM


def _part_major(b, n_tiles):
    # [E, n_tiles*128] f32 -> [E, 128, n_tiles], partition-major bias layout
    e = b.shape[0]
    return np.ascontiguousarray(
        b.reshape(e, n_tiles, 128).transpose(0, 2, 1)).astype(np.float32)


def _pack_xt(xs):
    # [B, E, F] -> [E, 128, F//128, B] (partition-major xT), fp8
    Bn, En, Fn = xs.shape
    xq = xs.astype(NPFP8)
    xt = xq.transpose(1, 2, 0).reshape(En, Fn // 128, 128, Bn)
    return np.ascontiguousarray(xt.transpose(0, 2, 1, 3))


def _pack_w0(W0):
    # [E, DF, F] f32 -> [E, F//256, 128, 2, DF] fp8 where
    # out[e, fb, p, i, k] = 64*W0[e, k, fb*256 + i*128 + p]
    E, _, F = W0.shape
    Wq = (W0 * WSCALE).astype(NPFP8)
    Wv = Wq.reshape(E, DF, F // 256, 2, 128).transpose(0, 2, 4, 3, 1)
    return np.ascontiguousarray(Wv)


def _pack_w1(W1):
    # [E, F, DF] f32 -> [E, KT//2, 128, 2, F] fp8 where
    # out[e, kb, p, i, f] = 64*W1[e, f, kb*256 + i*128 + p]
    E, F, _ = W1.shape
    Wq = (W1 * WSCALE).astype(NPFP8)
    Wv = Wq.reshape(E, F, KT // 2, 2, 128).transpose(0, 2, 4, 3, 1)
    return np.ascontiguousarray(Wv)


def _pack_core(c, x1, x2, W0_1, b0_1, W1_1, b1_1, W0_2, b0_2, W1_2, b1_2):
    i0, j0 = c * E1, c * E2
    s1, s2 = slice(i0, i0 + E1), slice(j0, j0 + E2)
    bb1 = _part_major(b0_1[s1], KT)
    bb2 = _part_major(b0_2[s2], KT)
    return {
        "xt1": _pack_xt(x1[:, s1, :]),
        "w0t1": _pack_w0(W0_1[s1]),
        "w1t1": _pack_w1(W1_1[s1]),
        "bb1": np.ascontiguousarray(bb1),
        "xt2": _pack_xt(x2[:, s2, :]),
        "w0t2": _pack_w0(W0_2[s2]),
        "w1t2": _pack_w1(W1_2[s2]),
        "bb2": np.ascontiguousarray(bb2),
    }


def run(inputs, trace=False):
    """Returns (out, BassKernelResults)."""
    x = np.asarray(inputs["x"], dtype=np.float32)
    x1 = x.reshape(B, D0, F1)
    x2 = np.ascontiguousarray(x.transpose(0, 2, 1, 3)).reshape(B, D1, F2)
    args = tuple(
        np.asarray(inputs[k], dtype=np.float32)
        for k in ("W0_1", "b0_1", "W1_1", "b1_1", "W0_2", "b0_2", "W1_2", "b1_2")
    )

    with ThreadPoolExecutor(max_workers=NCORES) as ex:
        in_maps = list(ex.map(lambda c: _pack_core(c, x1, x2, *args), range(NCORES)))
    nc = _get_program()
    res = run_bass_kernel_spmd(nc, in_maps, list(range(NCORES)), trace=trace)

    # outU [E, 2, 128, 4, 512]: region (p, j) = batch-half p, f-chunk j
    # -> u'[e, b, f]. outV [E, 1, 128, 4, 512]: region j = (bh, fc) with
    # fc = j % 2 -> v'[e, b, f]. Biases were dropped on-device; add the
    # 0.5*b1 terms here in fp32.
    U = np.concatenate([r["outU"] for r in res.results], axis=0).astype(np.float32)
    V = np.concatenate([r["outV"] for r in res.results], axis=0).astype(np.float32)
    U = U.transpose(0, 1, 3, 2, 4).reshape(D0, B, F1)
    V = V.reshape(D1, 128, 2, 2, 512).transpose(0, 2, 1, 3, 4).reshape(D1, B, F2)
    U = U + 0.5 * np.asarray(inputs["b1_1"], dtype=np.float32)[:, None, :]
    V = V + 0.5 * np.asarray(inputs["b1_2"], dtype=np.float32)[:, None, :]
    u_half = U.transpose(1, 0, 2).reshape(B, D0, D1, D2)
    v_half = V.transpose(1, 0, 2).reshape(B, D1, D0, D2).transpose(0, 2, 1, 3)
    out = x + u_half + v_half
    return np.ascontiguousarray(out, dtype=np.float32), res


def kernel(**inputs) -> np.ndarray:
    out, _ = run(inputs, trace=False)
    return out
